# revision 5
# baseline (speedup 1.0000x reference)
"""Trainium2 Bass kernel for nn_ContrastiveLearningLoss.

Strategy (data-parallel over the flattened region axis N = max_num*B = 40):
  - Each of 8 cores gets 5 regions of features_q/features_k reshaped to
    (40, 256, 11264) and mask reshaped to (40, 11264).
  - Masked sums s[n, c] = sum_hw f[n,c,hw]*m[n,hw] via one fused DVE
    scalar_tensor_tensor (out = (f*1)*mask, accum_out = row sums) per
    (stream, chunk), exactly as v1.
  - The mask row is broadcast to 128 partitions ON-CHIP instead of
    re-reading it 128x from HBM via SWDGE.  Per region the bf16 row
    (pre-cast on host, 22.5 KB) is loaded once via an HWDGE DMA issued on
    the ACT queue (keeping the SP FIFO pure feature bytes), then PE
    mat-muls ones[1,128]^T x row[1,512] into PSUM and ACT copies
    PSUM->SBUF f32.  This removes the 28.8 MB/core mask write traffic
    from the DMA rings (which are the bottleneck) and leaves the whole
    kernel HWDGE-only (no Pool/Q7 descriptor generation anywhere).
  - Schedule (TimelineSim): 2.3 us preamble + 320.7 us gap-free DMA
    stream (the 115.35 MB/core feature-byte floor at the model's
    360 GB/s) + 2.9 us drain + 1.4 us teardown = 327.6 us/core.
  - The tiny (40, 256) epilogue (means, normalize, 40x40 similarity, CE)
    runs on host in float32.
"""

import numpy as np

MAX_NUM, B, C, H, W = 10, 4, 256, 64, 176
HW = H * W          # 11264
N = MAX_NUM * B     # 40
N_CORES = 8
R = N // N_CORES    # 5 regions per core
TAU = 0.07
EPS = 1e-12

# per-region hw chunks; first/last small to shorten pipeline ramp/drain
FGROUPS = [(0, 4096), (4096, 3584), (7680, 2048), (9728, 1024), (10752, 512)]
N_CHUNKS = len(FGROUPS)
PSUM_W = 512        # one PSUM bank of f32 per broadcast chunk

_CACHE = {}


def _split_multi_waits(bir_bytes):
    """Legalize the BIR for this walrus build, which encodes at most ONE
    sync-wait per instruction: any instruction carrying N>1 waits gets N-1
    preceding same-engine Drain carriers, one wait each (same semantics —
    the engine executes them in order before the instruction)."""
    import json

    m = json.loads(bir_bytes)
    k = 0
    for fn in m["functions"]:
        for bb in fn["blocks"]:
            out = []
            for inst in bb["instructions"]:
                si = inst.get("sync_info")
                waits = (si or {}).get("on_wait") or []
                if len(waits) > 1:
                    for w in waits[:-1]:
                        k += 1
                        carrier = {
                            "engine": inst["engine"],
                            "ins": [],
                            "outs": [],
                            "name": f"{inst['name']}-sw{k}",
                            "opcode": "Drain",
                            "sync_info": {"on_update": [], "on_wait": [w]},
                        }
                        if "debug" in inst:
                            carrier["debug"] = inst["debug"]
                        out.append(carrier)
                    si["on_wait"] = [waits[-1]]
                out.append(inst)
            bb["instructions"] = out
    return json.dumps(m).encode()


def _build_bass(fbufs=6, mbufs=3, pbufs=4):
    import concourse.bass as bass
    import concourse.tile as tile
    from concourse import mybir

    nc = bass.Bass(trn_type="TRN2")
    f32 = mybir.dt.float32
    bf16 = mybir.dt.bfloat16
    fq = nc.dram_tensor("fq", (R, C, HW), f32, kind="ExternalInput")
    fk = nc.dram_tensor("fk", (R, C, HW), f32, kind="ExternalInput")
    # mask rows pre-cast to bf16 on host: keeps the whole kernel HWDGE-only
    # (no SWDGE ring init in the preamble, no gpsimd descriptor generation)
    mk = nc.dram_tensor("maskrow", (R, HW), bf16, kind="ExternalInput")
    out = nc.dram_tensor("out", (128, R * 4 * N_CHUNKS), f32, kind="ExternalOutput")

    with tile.TileContext(nc) as tc:
        with (
            tc.tile_pool(name="singles", bufs=1) as singles,
            tc.tile_pool(name="fpool", bufs=fbufs) as fpool,
            tc.tile_pool(name="mpool", bufs=mbufs) as mpool,
            tc.tile_pool(name="rowpool", bufs=2) as rowpool,
            tc.tile_pool(name="psum", bufs=pbufs, space="PSUM") as psum,
        ):
            acc = singles.tile([128, R * 4 * N_CHUNKS], f32, tag="acc")
            ones = singles.tile([1, 128], bf16, tag="ones")
            nc.vector.memset(ones[:, :], 1.0)

            srcs = [(fq, 0), (fq, 1), (fk, 0), (fk, 1)]
            for r in range(R):
                # region's bf16 mask row (22.5 KB): HWDGE load issued on the
                # ACT queue, keeping the SP FIFO pure feature bytes
                mrow = rowpool.tile([1, HW], bf16, tag="mrow", name="mrow")
                nc.scalar.dma_start(out=mrow[:, :], in_=mk[r:r + 1, :])
                for g, (goff, gw) in enumerate(FGROUPS):
                    mask_b = mpool.tile([128, 4096], f32, tag="mask_b", name="mask_b")
                    # broadcast row chunk to all 128 partitions via PE,
                    # PSUM -> SBUF f32 through the (otherwise idle) ACT engine
                    for off in range(0, gw, PSUM_W):
                        pw = min(PSUM_W, gw - off)
                        pt = psum.tile([128, PSUM_W], f32, tag="pt", name="pt")
                        nc.tensor.matmul(
                            pt[:, :pw],
                            ones[:, :],
                            mrow[:, goff + off:goff + off + pw],
                        )
                        nc.scalar.copy(out=mask_b[:, off:off + pw], in_=pt[:, :pw])
                    for s, (src, half) in enumerate(srcs):
                        ft = fpool.tile([128, 4096], f32, tag="f", name="ft")
                        nc.sync.dma_start(
                            out=ft[:, :gw],
                            in_=src[r, half * 128:(half + 1) * 128, goff:goff + gw],
                        )
                        col = (r * 4 + s) * N_CHUNKS + g
                        # out is written in-place into the f tile: its last
                        # writer is the same DMA the STT already waits on, so
                        # no extra WAW wait is generated.
                        nc.vector.scalar_tensor_tensor(
                            out=ft[:, :gw],
                            in0=ft[:, :gw],
                            scalar=1.0,
                            in1=mask_b[:, :gw],
                            op0=mybir.AluOpType.mult,
                            op1=mybir.AluOpType.mult,
                            accum_out=acc[:, col:col + 1],
                        )
            nc.sync.dma_start(out=out[:, :], in_=acc[:, :])

    orig_to_json = nc.to_json_bytes
    nc.to_json_bytes = lambda: _split_multi_waits(orig_to_json())
    return nc


def _get_bass():
    if "nc" not in _CACHE:
        _CACHE["nc"] = _build_bass()
    return _CACHE["nc"]


def _device_masked_sums(fq40, fk40, mk40, trace=False):
    """fq40/fk40: (40, 256, 11264) f32; mk40: (40, 11264) uint8.
    Returns sums_q, sums_k each (40, 256) f32 (and the run result object)."""
    from concourse import mybir
    from concourse.bass_utils import run_bass_kernel_spmd

    nc = _get_bass()
    np_bf16 = mybir.dt.np(mybir.dt.bfloat16)
    mkbf = mk40.astype(np_bf16)  # 0/1 exact in bf16
    in_maps = []
    for i in range(N_CORES):
        sl = slice(i * R, (i + 1) * R)
        in_maps.append({
            "fq": np.ascontiguousarray(fq40[sl]),
            "fk": np.ascontiguousarray(fk40[sl]),
            "maskrow": np.ascontiguousarray(mkbf[sl]),
        })
    res = run_bass_kernel_spmd(nc, in_maps, core_ids=list(range(N_CORES)), trace=trace)
    sums_q = np.empty((N, C), dtype=np.float32)
    sums_k = np.empty((N, C), dtype=np.float32)
    for i, r in enumerate(res.results):
        o = r["out"].reshape(128, R, 4, N_CHUNKS).sum(axis=3, dtype=np.float32)
        for rr in range(R):
            n = i * R + rr
            sums_q[n, 0:128] = o[:, rr, 0]
            sums_q[n, 128:256] = o[:, rr, 1]
            sums_k[n, 0:128] = o[:, rr, 2]
            sums_k[n, 128:256] = o[:, rr, 3]
    return sums_q, sums_k, res


def _epilogue(sums_q, sums_k, cnt):
    mean_q = sums_q / cnt[:, None]
    mean_k = sums_k / cnt[:, None]
    pad = mean_k[:, 0] != 0

    nrm_q = np.maximum(np.linalg.norm(mean_q, axis=-1, keepdims=True), EPS).astype(np.float32)
    nrm_k = np.maximum(np.linalg.norm(mean_k, axis=-1, keepdims=True), EPS).astype(np.float32)
    nq = mean_q / nrm_q
    nk = mean_k / nrm_k

    sim = (nk @ nq.T).astype(np.float32)
    logits = sim / np.float32(TAU)
    m = logits.max(axis=-1, keepdims=True)
    lse = np.log(np.exp(logits - m).sum(axis=-1, keepdims=True)).astype(np.float32) + m
    logp = logits - lse
    ce = -logp[np.arange(N), np.arange(N)]
    padf = pad.astype(np.float32)
    loss = (ce * padf).sum() / padf.sum()
    return np.asarray(loss, dtype=np.float32)


def kernel(features_q, features_k, mask, _trace=False, _ret_res=False):
    fq40 = np.asarray(features_q, dtype=np.float32).reshape(N, C, HW)
    fk40 = np.asarray(features_k, dtype=np.float32).reshape(N, C, HW)
    mk40 = np.asarray(mask).astype(np.uint8).reshape(N, HW)

    sums_q, sums_k, res = _device_masked_sums(fq40, fk40, mk40, trace=_trace)
    cnt = np.maximum(mk40.sum(axis=1, dtype=np.int64).astype(np.float32), np.float32(1.0))
    loss = _epilogue(sums_q, sums_k, cnt)
    if _ret_res:
        return loss, res
    return loss
